# revision 4
# baseline (speedup 1.0000x reference)
"""ColumnParallelLinearWithLoRA Trainium2 kernel.

Problem: out = x @ W^T + bias + per-token-LoRA, with
  x (4096, 4096) f32, W (4096, 4096) f32, bias (4096,) f32,
  lora_a (16, 16, 4096), lora_b (16, 4096, 16), indices (4096,) in [-1, 16).

Strategy (8 cores): row-parallel on tokens T — each core owns T/8 = 512
tokens end-to-end (base matmul + its own LoRA shrink/expand), which gives a
perfect FLOP split with zero replicated compute (vs. the column-parallel
hint, which replicates the shrink on every core).  The per-token LoRA gather
is reformulated as dense matmuls:

  tmpT[lr, t] = sum_h A_r[lr, h] * x[t, h]          (shrink; A_r = A.reshape(L*R, H))
  tmT         = tmpT * onehotT[lr, t]               (mask; onehot of indices, 0 for -1)
  out[t, o]   = sum_h x[t,h] W[o,h] + bias[o] + sum_lr tmT[lr,t] * B_r[lr,o]

where B_r[l*R+r, o] = lora_b[l, o, r].  All matmuls keep the same
orientation (stationary [K, t-or-lr], moving [K, o]), so no on-chip
transposes; operands are pre-transposed/tiled on the host.  bias is added
during the PSUM->SBUF eviction (DVE tensor_add with a host-replicated bias
tile).

Compute dtype bf16 (fp32 PSUM accumulate): fp32 matmul is 4 cycles/row on
TRN2 while bf16 is 1; measured L2 rel-err ~3e-3.
"""

import sys

sys.path.insert(0, "/opt/trn_rl_repo")

from contextlib import ExitStack

import numpy as np
import ml_dtypes

import concourse.bass as bass
import concourse.tile as tile
from concourse import bacc, mybir
from concourse.bass_utils import run_bass_kernel_spmd

T, H, O, L, R = 4096, 4096, 4096, 16, 16
N_CORES = 8
TS = T // N_CORES          # 512 tokens per core
P = 128
H_CHUNKS = H // P          # 32
O_SLICES = O // 512        # 8
T_TILES = TS // P          # 4
LR = L * R                 # 256
LR_TILES = LR // P         # 2
WG = 4                     # h-chunks per W DMA
XSPLIT = 4                 # x resident tiles
ASPLIT = 2                 # A resident tiles

BF16 = ml_dtypes.bfloat16
DT = mybir.dt.bfloat16
F32 = mybir.dt.float32


def build_program(repeats: int = 1):
    """Build + compile the per-core Bass program (same program on all cores)."""
    nc = bacc.Bacc("TRN2", debug=False, enable_asserts=False)

    xt = nc.dram_tensor("xt", [XSPLIT, P, (H_CHUNKS // XSPLIT) * TS], DT,
                        kind="ExternalInput").ap()
    # W^T, o-major tiling: [o_slice, h_chunk, 128, 512]
    wt = nc.dram_tensor("wt", [O_SLICES, H_CHUNKS, P, 512], DT,
                        kind="ExternalInput").ap()
    at = nc.dram_tensor("at", [ASPLIT, P, (H_CHUNKS // ASPLIT) * LR], DT,
                        kind="ExternalInput").ap()
    bt = nc.dram_tensor("bt", [LR_TILES, P, O], DT, kind="ExternalInput").ap()
    mk = nc.dram_tensor("mk", [LR_TILES, P, TS], DT, kind="ExternalInput").ap()
    bs = nc.dram_tensor("bs", [P, O], DT, kind="ExternalInput").ap()
    out = nc.dram_tensor("out", [TS, O], F32, kind="ExternalOutput").ap()

    XC = H_CHUNKS // XSPLIT  # h-chunks per x tile (8)
    AC = H_CHUNKS // ASPLIT  # h-chunks per A tile (16)

    with tile.TileContext(nc) as tc, ExitStack() as ctx:
        const = ctx.enter_context(tc.tile_pool(name="const", bufs=1))
        psum = ctx.enter_context(tc.tile_pool(name="psum", bufs=8, space="PSUM"))
        wpool = ctx.enter_context(tc.tile_pool(name="wpool", bufs=4))
        opool = ctx.enter_context(tc.tile_pool(name="opool", bufs=6))

        for _rep in range(repeats):
            # resident inputs (split loads so the PE can start early)
            x_sb = []
            for i in range(XSPLIT):
                x_t = const.tile([P, XC * TS], DT, tag=f"x{i}")
                nc.sync.dma_start(x_t[:], xt[i][:])
                x_sb.append(x_t)
            a_sb = []
            for i in range(ASPLIT):
                a_t = const.tile([P, AC * LR], DT, tag=f"a{i}")
                nc.sync.dma_start(a_t[:], at[i][:])
                a_sb.append(a_t)
            b_sb = []
            for lt in range(LR_TILES):
                b_t = const.tile([P, O], DT, tag=f"b{lt}")
                nc.sync.dma_start(b_t[:], bt[lt][:])
                b_sb.append(b_t)
            m_sb = []
            for lt in range(LR_TILES):
                m_t = const.tile([P, TS], DT, tag=f"m{lt}")
                nc.sync.dma_start(m_t[:], mk[lt][:])
                m_sb.append(m_t)
            bias_sb = const.tile([P, O], DT, tag="bias")
            nc.sync.dma_start(bias_sb[:], bs[:])

            tm_sb = const.tile([P, LR_TILES * TS], DT, tag="tm")

            def x_chunk(c):
                return x_sb[c // XC][:, (c % XC) * TS : (c % XC + 1) * TS]

            def a_chunk(c, lt):
                base = (c % AC) * LR + lt * P
                return a_sb[c // AC][:, base : base + P]

            # ---- shrink: tmT[lr, t] = sum_h A_r[lr, h] x[t, h], then mask ----
            for lt in range(LR_TILES):
                ps_s = psum.tile([P, TS], F32, tag="ps")
                for c in range(H_CHUNKS):
                    nc.tensor.matmul(
                        ps_s[:],
                        lhsT=a_chunk(c, lt),
                        rhs=x_chunk(c),
                        start=(c == 0),
                        stop=(c == H_CHUNKS - 1),
                    )
                nc.vector.tensor_mul(
                    tm_sb[:, lt * TS : (lt + 1) * TS], ps_s[:], m_sb[lt][:]
                )

            # ---- main: out[t, o] = x@W^T + tmT^T @ B_r (+bias at evict) ----
            for o_i in range(O_SLICES):
                osl = slice(o_i * 512, (o_i + 1) * 512)
                pts = [
                    psum.tile([P, 512], F32, tag="ps", name=f"pt_{o_i}_{tt}")
                    for tt in range(T_TILES)
                ]
                for g in range(H_CHUNKS // WG):
                    w_t = wpool.tile([P, WG * 512], DT, tag="w")
                    nc.sync.dma_start(
                        w_t.rearrange("p (g f) -> p g f", g=WG),
                        wt[o_i, g * WG : (g + 1) * WG].rearrange("g p f -> p g f"),
                    )
                    for gi in range(WG):
                        c = g * WG + gi
                        for tt in range(T_TILES):
                            nc.tensor.matmul(
                                pts[tt][:],
                                lhsT=x_chunk(c)[:, tt * P : (tt + 1) * P],
                                rhs=w_t[:, gi * 512 : (gi + 1) * 512],
                                start=(c == 0),
                                stop=False,
                            )
                for lt in range(LR_TILES):
                    for tt in range(T_TILES):
                        nc.tensor.matmul(
                            pts[tt][:],
                            lhsT=tm_sb[:, lt * TS + tt * P : lt * TS + (tt + 1) * P],
                            rhs=b_sb[lt][:, osl],
                            start=False,
                            stop=(lt == LR_TILES - 1),
                        )
                for tt in range(T_TILES):
                    o_t = opool.tile([P, 512], F32, tag="o")
                    nc.vector.tensor_add(o_t[:], pts[tt][:], bias_sb[:, osl])
                    nc.sync.dma_start(out[tt * P : (tt + 1) * P, osl], o_t[:])

    nc.compile()
    return nc


def prep_inputs(x, weight, bias, lora_a_stacked, lora_b_stacked, indices):
    """Host-side shard + layout prep. Returns per-core input maps."""
    x = np.asarray(x, dtype=np.float32)
    weight = np.asarray(weight, dtype=np.float32)
    bias = np.asarray(bias, dtype=np.float32)
    lora_a = np.asarray(lora_a_stacked, dtype=np.float32)
    lora_b = np.asarray(lora_b_stacked, dtype=np.float32)
    indices = np.asarray(indices)

    # W^T tiled o-major: (H, O) -> (O_SLICES, H_CHUNKS, 128, 512)
    wtb = weight.T.astype(BF16)  # (H, O)
    w_pre = np.ascontiguousarray(
        wtb.reshape(H_CHUNKS, P, O_SLICES, 512).transpose(2, 0, 1, 3)
    )

    # A_r^T: (H, LR) -> [128, c*LR + lr] layout, split into ASPLIT tiles
    a_rt = lora_a.reshape(LR, H).T.astype(BF16)  # (H, LR)
    a_pre = np.ascontiguousarray(
        a_rt.reshape(ASPLIT, H_CHUNKS // ASPLIT, P, LR).transpose(0, 2, 1, 3)
    ).reshape(ASPLIT, P, (H_CHUNKS // ASPLIT) * LR)

    # B_r: lora_b (L, O, R) -> B_r[l*R+r, o] -> (LR_TILES, 128, O)
    b_r = np.ascontiguousarray(lora_b.transpose(0, 2, 1)).reshape(LR, O).astype(BF16)
    b_pre = np.ascontiguousarray(b_r.reshape(LR_TILES, P, O))

    bias_pre = np.ascontiguousarray(
        np.broadcast_to(bias.astype(BF16)[None, :], (P, O))
    )

    in_maps = []
    for c in range(N_CORES):
        xs = x[c * TS : (c + 1) * TS, :]  # (TS, H)
        xts = xs.T.astype(BF16)  # (H, TS)
        x_pre = np.ascontiguousarray(
            xts.reshape(XSPLIT, H_CHUNKS // XSPLIT, P, TS).transpose(0, 2, 1, 3)
        ).reshape(XSPLIT, P, (H_CHUNKS // XSPLIT) * TS)

        idx_s = indices[c * TS : (c + 1) * TS]
        onehot = (idx_s[None, :] == np.arange(L)[:, None]).astype(BF16)  # (L, TS)
        mk_pre = np.ascontiguousarray(
            np.repeat(onehot, R, axis=0).reshape(LR_TILES, P, TS)
        )

        in_maps.append(
            {
                "xt": x_pre,
                "wt": w_pre,
                "at": a_pre,
                "bt": b_pre,
                "mk": mk_pre,
                "bs": bias_pre,
            }
        )
    return in_maps


_PROGRAM_CACHE = {}


def kernel(x, weight, bias, lora_a_stacked, lora_b_stacked, indices):
    if "nc" not in _PROGRAM_CACHE:
        _PROGRAM_CACHE["nc"] = build_program()
    nc = _PROGRAM_CACHE["nc"]
    in_maps = prep_inputs(x, weight, bias, lora_a_stacked, lora_b_stacked, indices)
    res = run_bass_kernel_spmd(nc, in_maps, list(range(N_CORES)))
    return np.concatenate([res.results[c]["out"] for c in range(N_CORES)], axis=0)


# revision 12
# speedup vs baseline: 4.0997x; 4.0997x over previous
"""ColumnParallelLinearWithLoRA Trainium2 kernel.

Problem: out = x @ W^T + bias + per-token-LoRA, with
  x (4096, 4096) f32, W (4096, 4096) f32, bias (4096,) f32,
  lora_a (16, 16, 4096), lora_b (16, 4096, 16), indices (4096,) in [-1, 16).

Strategy (8 cores): row-parallel on tokens T — each core owns T/8 = 512
tokens end-to-end (base matmul + its own LoRA shrink/expand), which gives a
perfect FLOP split with zero replicated compute (vs. the column-parallel
hint, which replicates the shrink on every core).  The per-token LoRA gather
is reformulated as dense matmuls:

  tmpT[lr, t] = sum_h A_r[lr, h] * x[t, h]          (shrink; A_r = A.reshape(L*R, H))
  tmT         = tmpT * onehotT[lr, t]               (mask; onehot of indices, 0 for -1)
  out[t, o]   = sum_h x[t,h] W[o,h] + bias[o] + sum_lr tmT[lr,t] * B_r[lr,o]

where B_r[l*R+r, o] = lora_b[l, o, r].  All matmuls keep the same
orientation (stationary [K, t-or-lr], moving [K, o]), so no on-chip
transposes; operands are pre-transposed/tiled on the host.  bias is added
during the PSUM->SBUF eviction (DVE tensor_add with a host-replicated bias
tile).

Compute dtype bf16 (fp32 PSUM accumulate): fp32 matmul is 4 cycles/row on
TRN2 while bf16 is 1; measured L2 rel-err ~3e-3.
"""

import sys

sys.path.insert(0, "/opt/trn_rl_repo")

from contextlib import ExitStack

import numpy as np
import ml_dtypes

import concourse.bass as bass
import concourse.tile as tile
from concourse import bacc, mybir
from concourse.bass_utils import run_bass_kernel_spmd

T, H, O, L, R = 4096, 4096, 4096, 16, 16
N_CORES = 8
TS = T // N_CORES          # 512 tokens per core
P = 128
H_CHUNKS = H // P          # 32
O_SLICES = O // 512        # 8
T_TILES = TS // P          # 4
LR = L * R                 # 256
LR_TILES = LR // P         # 2
WG = 4                     # h-chunks per W DMA
_WPOOL_BUFS = 6            # W streaming double-buffer depth
XSPLIT = 4                 # x resident tiles
ASPLIT = 2                 # A resident tiles

BF16 = ml_dtypes.bfloat16
F32 = mybir.dt.float32

# compute dtype: "bf16" or "f32r" (fp32 data, full-rate PE mode)
COMPUTE_DTYPE = "bf16"


def build_program(repeats: int = 1, compute_dtype: str | None = None):
    """Build + compile the per-core Bass program (same program on all cores)."""
    DT = mybir.dt.bfloat16 if (compute_dtype or COMPUTE_DTYPE) == "bf16" else mybir.dt.float32r
    nc = bacc.Bacc("TRN2", debug=False, enable_asserts=False)

    xt = nc.dram_tensor("xt", [XSPLIT, P, (H_CHUNKS // XSPLIT) * TS], DT,
                        kind="ExternalInput").ap()
    # W^T, o-major tiling: [o_slice, h_chunk, 128, 512]
    wt = nc.dram_tensor("wt", [O_SLICES, H_CHUNKS, P, 512], DT,
                        kind="ExternalInput").ap()
    at = nc.dram_tensor("at", [ASPLIT, P, (H_CHUNKS // ASPLIT) * LR], DT,
                        kind="ExternalInput").ap()
    bt = nc.dram_tensor("bt", [LR_TILES, P, O], DT, kind="ExternalInput").ap()
    mk = nc.dram_tensor("mk", [LR_TILES, P, TS], DT, kind="ExternalInput").ap()
    bs = nc.dram_tensor("bs", [P, O], DT, kind="ExternalInput").ap()
    out = nc.dram_tensor("out", [TS, O], F32, kind="ExternalOutput").ap()

    XC = H_CHUNKS // XSPLIT  # h-chunks per x tile (8)
    AC = H_CHUNKS // ASPLIT  # h-chunks per A tile (16)

    with tile.TileContext(nc) as tc, ExitStack() as ctx:
        const = ctx.enter_context(tc.tile_pool(name="const", bufs=1))
        psum = ctx.enter_context(tc.tile_pool(name="psum", bufs=8, space="PSUM"))
        wpool = ctx.enter_context(tc.tile_pool(name="wpool", bufs=_WPOOL_BUFS))
        opool = ctx.enter_context(tc.tile_pool(name="opool", bufs=6))

        for _rep in range(repeats):
            # Resident-tile handles, loaded lazily.  The HWDGE sync ring
            # drains in trace order, so loads are issued in deadline order:
            # x0 + o-slice-0's first W group gate the very first matmuls;
            # the remaining x tiles, mask, and A ride between W groups; B and
            # bias (needed only at expand/evict of o-slice 0) come last.
            x_sb = [None] * XSPLIT
            a_sb = [None] * ASPLIT
            m_sb = [None] * LR_TILES
            b_sb = [None] * LR_TILES
            bias_ref = [None]
            tm_sb = const.tile([P, LR_TILES * TS], DT, tag="tm")

            def dma_x(i):
                x_t = const.tile([P, XC * TS], DT, tag=f"x{i}", name=f"x_t{i}")
                nc.sync.dma_start(x_t[:], xt[i][:])
                x_sb[i] = x_t

            def dma_a(i):
                a_t = const.tile([P, AC * LR], DT, tag=f"a{i}", name=f"a_t{i}")
                nc.sync.dma_start(a_t[:], at[i][:])
                a_sb[i] = a_t

            def dma_m(lt):
                m_t = const.tile([P, TS], DT, tag=f"m{lt}", name=f"m_t{lt}")
                nc.sync.dma_start(m_t[:], mk[lt][:])
                m_sb[lt] = m_t

            def dma_b(lt):
                b_t = const.tile([P, O], DT, tag=f"b{lt}", name=f"b_t{lt}")
                nc.sync.dma_start(b_t[:], bt[lt][:])
                b_sb[lt] = b_t

            def dma_bias():
                bias_sb = const.tile([P, O], DT, tag="bias")
                nc.sync.dma_start(bias_sb[:], bs[:])
                bias_ref[0] = bias_sb

            def x_chunk(c):
                return x_sb[c // XC][:, (c % XC) * TS : (c % XC + 1) * TS]

            def a_chunk(c, lt):
                base = (c % AC) * LR + lt * P
                return a_sb[c // AC][:, base : base + P]

            # late-load plan for o-slice 0, keyed by W-group index
            extras = {
                1: [lambda: dma_x(1)],
                2: [lambda: dma_x(2)],
                3: [lambda: dma_x(3)],
                4: [lambda: dma_m(0), lambda: dma_m(1)],
                5: [lambda: dma_a(0)],
                6: [lambda: dma_a(1)],
            }

            def emit_base(o_i):
                """bias-free base accumulation for one o-slice; returns psum tiles"""
                pts = [
                    psum.tile([P, 512], F32, tag="ps", name=f"pt_{o_i}_{tt}")
                    for tt in range(T_TILES)
                ]
                for g in range(H_CHUNKS // WG):
                    w_t = wpool.tile([P, WG * 512], DT, tag="w")
                    nc.sync.dma_start(
                        w_t.rearrange("p (g f) -> p g f", g=WG),
                        wt[o_i, g * WG : (g + 1) * WG].rearrange("g p f -> p g f"),
                    )
                    if o_i == 0:
                        for fn in extras.get(g, []):
                            fn()
                    for gi in range(WG):
                        c = g * WG + gi
                        for tt in range(T_TILES):
                            nc.tensor.matmul(
                                pts[tt][:],
                                lhsT=x_chunk(c)[:, tt * P : (tt + 1) * P],
                                rhs=w_t[:, gi * 512 : (gi + 1) * 512],
                                start=(c == 0),
                                stop=False,
                            )
                return pts

            def emit_expand_evict(o_i, pts):
                osl = slice(o_i * 512, (o_i + 1) * 512)
                for lt in range(LR_TILES):
                    for tt in range(T_TILES):
                        nc.tensor.matmul(
                            pts[tt][:],
                            lhsT=tm_sb[:, lt * TS + tt * P : lt * TS + (tt + 1) * P],
                            rhs=b_sb[lt][:, osl],
                            start=False,
                            stop=(lt == LR_TILES - 1),
                        )
                for tt in range(T_TILES):
                    o_t = opool.tile([P, 512], F32, tag="o")
                    nc.vector.tensor_add(o_t[:], pts[tt][:], bias_ref[0][:, osl])
                    nc.sync.dma_start(out[tt * P : (tt + 1) * P, osl], o_t[:])

            # ---- o-slice 0 base first: PE starts after x0 + W group 0 ----
            dma_x(0)
            pts0 = emit_base(0)
            for lt in range(LR_TILES):
                dma_b(lt)
            dma_bias()

            # ---- shrink: tmT[lr, t] = sum_h A_r[lr, h] x[t, h], then mask ----
            for lt in range(LR_TILES):
                ps_s = psum.tile([P, TS], F32, tag="ps")
                for c in range(H_CHUNKS):
                    nc.tensor.matmul(
                        ps_s[:],
                        lhsT=a_chunk(c, lt),
                        rhs=x_chunk(c),
                        start=(c == 0),
                        stop=(c == H_CHUNKS - 1),
                    )
                nc.vector.tensor_mul(
                    tm_sb[:, lt * TS : (lt + 1) * TS], ps_s[:], m_sb[lt][:]
                )

            # ---- finish o-slice 0, then the rest ----
            emit_expand_evict(0, pts0)
            for o_i in range(1, O_SLICES):
                pts = emit_base(o_i)
                emit_expand_evict(o_i, pts)

    nc.compile()
    return nc


def prep_inputs(x, weight, bias, lora_a_stacked, lora_b_stacked, indices,
                compute_dtype: str | None = None):
    """Host-side shard + layout prep. Returns per-core input maps."""
    BF16 = ml_dtypes.bfloat16 if (compute_dtype or COMPUTE_DTYPE) == "bf16" else np.float32
    x = np.asarray(x, dtype=np.float32)
    weight = np.asarray(weight, dtype=np.float32)
    bias = np.asarray(bias, dtype=np.float32)
    lora_a = np.asarray(lora_a_stacked, dtype=np.float32)
    lora_b = np.asarray(lora_b_stacked, dtype=np.float32)
    indices = np.asarray(indices)

    # W^T tiled o-major: (H, O) -> (O_SLICES, H_CHUNKS, 128, 512)
    wtb = weight.T.astype(BF16)  # (H, O)
    w_pre = np.ascontiguousarray(
        wtb.reshape(H_CHUNKS, P, O_SLICES, 512).transpose(2, 0, 1, 3)
    )

    # A_r^T: (H, LR) -> [128, c*LR + lr] layout, split into ASPLIT tiles
    a_rt = lora_a.reshape(LR, H).T.astype(BF16)  # (H, LR)
    a_pre = np.ascontiguousarray(
        a_rt.reshape(ASPLIT, H_CHUNKS // ASPLIT, P, LR).transpose(0, 2, 1, 3)
    ).reshape(ASPLIT, P, (H_CHUNKS // ASPLIT) * LR)

    # B_r: lora_b (L, O, R) -> B_r[l*R+r, o] -> (LR_TILES, 128, O)
    b_r = np.ascontiguousarray(lora_b.transpose(0, 2, 1)).reshape(LR, O).astype(BF16)
    b_pre = np.ascontiguousarray(b_r.reshape(LR_TILES, P, O))

    bias_pre = np.ascontiguousarray(
        np.broadcast_to(bias.astype(BF16)[None, :], (P, O))
    )

    in_maps = []
    for c in range(N_CORES):
        xs = x[c * TS : (c + 1) * TS, :]  # (TS, H)
        xts = xs.T.astype(BF16)  # (H, TS)
        x_pre = np.ascontiguousarray(
            xts.reshape(XSPLIT, H_CHUNKS // XSPLIT, P, TS).transpose(0, 2, 1, 3)
        ).reshape(XSPLIT, P, (H_CHUNKS // XSPLIT) * TS)

        idx_s = indices[c * TS : (c + 1) * TS]
        onehot = (idx_s[None, :] == np.arange(L)[:, None]).astype(BF16)  # (L, TS)
        mk_pre = np.ascontiguousarray(
            np.repeat(onehot, R, axis=0).reshape(LR_TILES, P, TS)
        )

        in_maps.append(
            {
                "xt": x_pre,
                "wt": w_pre,
                "at": a_pre,
                "bt": b_pre,
                "mk": mk_pre,
                "bs": bias_pre,
            }
        )
    return in_maps


_PROGRAM_CACHE = {}


def kernel(x, weight, bias, lora_a_stacked, lora_b_stacked, indices):
    if "nc" not in _PROGRAM_CACHE:
        _PROGRAM_CACHE["nc"] = build_program()
    nc = _PROGRAM_CACHE["nc"]
    in_maps = prep_inputs(x, weight, bias, lora_a_stacked, lora_b_stacked, indices)
    res = run_bass_kernel_spmd(nc, in_maps, list(range(N_CORES)))
    return np.concatenate([res.results[c]["out"] for c in range(N_CORES)], axis=0)
